# revision 86
# baseline (speedup 1.0000x reference)
"""Causal self-attention (B=2, T=2048, C=1024, H=16, D=64) on 8 trn2 cores.

Sharding: core c handles batch b = c//4 and head group hg = c%4 (heads
4*hg .. 4*hg+3).  Each core computes q/k/v projections for its 4 heads
(as 2 stacked head pairs), causal-softmax attention, and a partial
output projection y_partial = O_heads @ Wo[:, heads].T.  The host sums
the 4 fp16 partials per batch and adds the bias.

Mixed precision (fp32 PSUM accumulation everywhere):
  Q,K path fp8 e4m3: weights pre-scaled by 32 on host (exact power of
  2, folded into the softmax scale 2^-15), x cast to e4m3.  QK
  projections are DoubleRow fp8 matmuls (K=256/pass, 0.5 cyc/row).
  S also runs DoubleRow for I>=1: a 4-DMA on-chip repack reshuffles
  q/k from [128 feat, 512 tok] to [32h+p, i, tp, key] so the K=64
  contraction packs as [32, 2].  fp8 logit noise (~0.02) averages out
  in the softmax sum (measured rel err 1.2e-2 vs the 2e-2 gate).
  V path bf16 (V errors land 1:1 in concentrated-attention rows; fp8
  V measures 2.6e-2 and fails): V projection, transpose, vaug, P, O
  matmul and the y projection run bf16; y partials stream out fp16.

Schedule: the kernel is ACT(exp)-bound (~82us of exp on a ~116us
critical path), so emission is built around keeping the exp stream
dense.  Each attention(I) is a lagged software pipeline: the O stream
trails S/exp by `lag` jb-steps (p_sb tiles park in ppool) so O-side
stalls (vaug, oT ring, normalize) never starve ACT; when an
attention's SE stream drains, it pre-issues the next attention's
first SE steps (next_se) to bridge the phase boundary.  Projection /
transpose / y-projection work rides as small PE filler units popped
between attention steps, sized so they fit the per-step PE slack.
DMA issue costs ~650ns of serial SP-SEQ time per transfer, so x
chunks load as one rearranged DMA per dtype and the SP queue order is
hand-placed (bulk xb transfers must not sit ahead of the small
S-gating repacks).  Non-exp elementwise work spreads over DVE and
Pool (gpsimd cannot read PSUM); the y tail uses the idle ACT hwdge
queue and the freed ps_s banks.
"""
import numpy as np
import ml_dtypes

import concourse.tile as tile
import concourse.mybir as mybir
from concourse import bacc
from concourse.bass_utils import run_bass_kernel_spmd

FP = mybir.dt.float32
BF = mybir.dt.bfloat16
F8 = mybir.dt.float8e4
F16 = mybir.dt.float16
NPBF = ml_dtypes.bfloat16
NPF8 = ml_dtypes.float8_e4m3
DR = mybir.MatmulPerfMode.DoubleRow
B, T, C = 2, 2048, 1024
H, D = 16, 64
SCALE = 1.0 / 32.0 / 1024.0  # 1/sqrt(C), folded with the 32x W pre-scale
N_CORES = 8
NKT = C // 128  # 8 K-tiles over the embedding dim
NTK = T // 128  # 16 Tk tiles
NI = T // 512  # 4 Tq chunks
EXP = mybir.ActivationFunctionType.Exp

_nc_cache = {}


def build_kernel(repeats=1):
    key = repeats
    if key in _nc_cache:
        return _nc_cache[key]

    nc = bacc.Bacc("TRN2", target_bir_lowering=False, debug=False)

    xT8_d = nc.dram_tensor("xT8", [C, T], F8, kind="ExternalInput").ap()
    xTb_d = nc.dram_tensor("xTb", [C, T], BF, kind="ExternalInput").ap()
    wq8_d = nc.dram_tensor("wq8", [C, 256], F8, kind="ExternalInput").ap()
    wk8_d = nc.dram_tensor("wk8", [C, 256], F8, kind="ExternalInput").ap()
    wvb_d = nc.dram_tensor("wvb", [C, 256], BF, kind="ExternalInput").ap()
    wob_d = nc.dram_tensor("wob", [256, C], BF, kind="ExternalInput").ap()
    y_d = nc.dram_tensor("y", [T, C], F16, kind="ExternalOutput").ap()

    # mask_big[p, y] = 1 iff y >= p + 384 : slice [., c0:c0+z+128] with
    # c0 = 384 - z, z = j*128 - I*512 masks diagonal tile j (cols < z are
    # fully below the causal boundary, the next 128 are triangular)
    mask_np = (
        np.arange(896)[None, :] >= (np.arange(128)[:, None] + 384)
    ).astype(NPBF)
    mask_d = nc.inline_tensor(mask_np, "mask_big").ap()
    ident_d = nc.inline_tensor(np.eye(128, dtype=NPBF), "ident").ap()
    ones_d = nc.inline_tensor(np.ones((128, 2), dtype=NPBF), "ones").ap()

    with tile.TileContext(nc) as tc:
        with (
            tc.tile_pool(name="persist", bufs=1) as pp,
            tc.tile_pool(name="xpool", bufs=3) as xpool,
            tc.tile_pool(name="ppool", bufs=18) as ppool,
            tc.tile_pool(name="spool", bufs=4) as spool,
            tc.tile_pool(name="ypool", bufs=4) as ypool,
            tc.tile_pool(name="ps_s", bufs=2, space="PSUM") as ps_s,
            tc.tile_pool(name="ps_o", bufs=2, space="PSUM") as ps_o,
            tc.tile_pool(name="ps_y", bufs=2, space="PSUM") as ps_y,
        ):
            # ---- critical-path DMAs first: wq, then x chunk 0, then the
            # rest; weight matrices load as ONE rearranged DMA each ----
            def dma_x8(c, nm):
                # whole 512-token chunk of fp8 x in ONE DMA: [128, 8, 512]
                xt = xpool.tile([128, NKT, 512], F8, tag="xt8", name=nm)
                nc.sync.dma_start(
                    xt[:],
                    xT8_d[:, c * 512 : c * 512 + 512].rearrange(
                        "(n p) d -> p n d", p=128
                    ),
                )
                return xt

            def dma_xb(c, nm):
                xt = xpool.tile([128, NKT, 512], BF, tag="xtb", name=nm)
                nc.sync.dma_start(
                    xt[:],
                    xTb_d[:, c * 512 : c * 512 + 512].rearrange(
                        "(n p) d -> p n d", p=128
                    ),
                )
                return xt

            # lead-in: split the first weight/x transfers so the very first
            # q-pair0 projection (and with it S(0) -> exp(0)) starts ASAP
            wq_big = pp.tile([128, NKT, 256], F8, tag="wq")
            nc.sync.dma_start(
                wq_big[:, :, 0:128],
                wq8_d[:, 0:128].rearrange("(n p) d -> p n d", p=128),
            )
            x8_by_chunk = {0: None, 1: None}
            xb_by_chunk = {0: None, 1: None}
            x8_by_chunk[0] = dma_x8(0, "x8c0")
            wk_big = pp.tile([128, NKT, 256], F8, tag="wk")
            nc.sync.dma_start(
                wk_big[:, :, 0:128],
                wk8_d[:, 0:128].rearrange("(n p) d -> p n d", p=128),
            )
            nc.sync.dma_start(
                wq_big[:, :, 128:256],
                wq8_d[:, 128:256].rearrange("(n p) d -> p n d", p=128),
            )
            nc.sync.dma_start(
                wk_big[:, :, 128:256],
                wk8_d[:, 128:256].rearrange("(n p) d -> p n d", p=128),
            )
            x8_by_chunk[1] = dma_x8(1, "x8c1")
            wv_big = pp.tile([128, NKT, 256], BF, tag="wv")
            nc.sync.dma_start(
                wv_big[:, :, :], wvb_d.rearrange("(n p) d -> p n d", p=128)
            )
            # the rest of the first-pass DMAs (mask/ident/xb/wo and the q/k
            # DR repacks) are emitted inside the rep-0 flow so their SP
            # queue positions line up with when each consumer needs them
            mask = pp.tile([128, 896], BF, tag="mask")
            ones_sb = pp.tile([128, 2], BF, tag="ones")
            ident = pp.tile([128, 128], BF, tag="ident")
            wo_big = pp.tile([128, 2, C], BF, tag="wo")
            wo = [wo_big[:, kk, :] for kk in range(2)]

            # ---- persistent activations, chunked per 512 columns ----
            # combined q/k staging: [:, 0+pair, :] = q, [:, 2+pair, :] = k
            qkTc = [
                pp.tile([128, 4, 512], F8, tag=f"qkT{i}", name=f"qkT{i}")
                for i in range(NI)
            ]
            vTc = [
                [pp.tile([128, 512], BF, tag=f"vT{p}_{i}", name=f"vT{p}_{i}")
                 for i in range(NI)]
                for p in range(2)
            ]
            otstc = [
                [pp.tile([128, 512], BF, tag=f"ot{p}_{i}", name=f"otst{p}_{i}")
                 for i in range(NI)]
                for p in range(2)
            ]
            # DoubleRow repack of q/k: [32h+p, i, tp, key] =
            # qkTc[64h+32i+p, tp, key], so S runs as one fp8 DR matmul per
            # (j, head) at 0.5 cyc/row.  4 flat DMAs per chunk.
            qkdr = [
                pp.tile([64, 2, 4, 512], F8, tag=f"qkdr{i}", name=f"qkdr{i}")
                for i in range(NI)
            ]
            vaug = [
                [
                    pp.tile([128, 130], BF, tag=f"va{p}_{t}", name=f"vaug{p}_{t}")
                    for t in range(NTK)
                ]
                for p in range(2)
            ]

            # ---- emission helpers ----
            R = [0]

            def emit_xt_chunk(c):
                return dma_x8(c, f"x8c{c}_r{R[0]}"), dma_xb(c, f"xbc{c}_r{R[0]}")

            def emit_proj_qk1(wts, tp_base, nm, c, pair, x8t):
                # one (tensor, pair) QK unit: DoubleRow fp8, K=256 per matmul
                ps = ps_y.tile([128, 512], FP, tag="ps_proj",
                               name=f"pspr{nm}{c}_{pair}_r{R[0]}")
                for kp in range(4):
                    nc.tensor.matmul(
                        ps[:],
                        lhsT=wts[
                            :, 2 * kp : 2 * kp + 2, pair * 128 : pair * 128 + 128
                        ],
                        rhs=x8t[:, 2 * kp : 2 * kp + 2, :],
                        start=(kp == 0),
                        stop=(kp == 3),
                        perf_mode=DR,
                    )
                nc.vector.tensor_copy(qkTc[c][:, tp_base + pair, :], ps[:])

            def emit_repack(c, k_only=False):
                # shuffle q/k staging into the DR layout: 4 flat DMAs
                tp0 = 2 if k_only else 0
                for g in range(2):
                    for i in range(2):
                        nc.sync.dma_start(
                            qkdr[c][32 * g : 32 * g + 32, i, tp0:4, :],
                            qkTc[c][
                                64 * g + 32 * i : 64 * g + 32 * i + 32,
                                tp0:4, :,
                            ],
                        )

            def emit_proj_qk(c, x8t):
                # pair-0 q,k first so the first S matmuls unblock earliest
                for pair in range(2):
                    for wts, tpb, nm in ((wq_big, 0, "q"), (wk_big, 2, "k")):
                        emit_proj_qk1(wts, tpb, nm, c, pair, x8t)

            def emit_proj_v1(c, pair, xbt, half=None):
                # one V-pair unit: bf16, K=128 per matmul (8 passes); with
                # half=0/1 emits the low/high 4 K-tiles (finer filler grain)
                if half in (None, 0):
                    ps = ps_y.tile([128, 512], FP, tag="ps_proj",
                                   name=f"psprv{c}_{pair}_r{R[0]}")
                    vps_open[(c, pair)] = ps
                else:
                    ps = vps_open.pop((c, pair))
                kks = range(NKT) if half is None else range(4 * half, 4 * half + 4)
                for kk in kks:
                    nc.tensor.matmul(
                        ps[:],
                        lhsT=wv_big[:, kk, pair * 128 : pair * 128 + 128],
                        rhs=xbt[:, kk, :],
                        start=(kk == 0),
                        stop=(kk == NKT - 1),
                        skip_group_check=(half is not None),
                    )
                if half in (None, 1):
                    nc.vector.tensor_copy(vTc[pair][c][:], ps[:])

            vps_open = {}

            def emit_proj_v(c, xbt):
                for pair in range(2):
                    emit_proj_v1(c, pair, xbt)

            def emit_transpose1(pair, t):
                c = t // 4
                pst = ps_y.tile([128, 128], BF, tag="ps_proj",
                                name=f"pstr{pair}_{t}_r{R[0]}")
                nc.tensor.transpose(
                    pst[:],
                    vTc[pair][c][:, (t % 4) * 128 : (t % 4) * 128 + 128],
                    ident[:],
                )
                va = vaug[pair][t]
                # both heads' 64-wide blocks in one strided copy (DVE: the
                # Pool engine cannot read PSUM), ones-columns on Pool
                nc.vector.tensor_copy(
                    va[:].rearrange("p (two f) -> p two f", two=2)[:, :, 0:64],
                    pst[:].rearrange("p (two f) -> p two f", two=2),
                )
                nc.gpsimd.tensor_copy(
                    va[:].rearrange("p (two f) -> p two f", two=2)[:, :, 64:65],
                    ones_sb[:].rearrange("p (two f) -> p two f", two=2),
                )

            def emit_transposes(c):
                for pair in range(2):
                    for t in range(4 * c, 4 * c + 4):
                        emit_transpose1(pair, t)

            fillers = []

            def emit_yproj_chunk(t, tail=False):
                # both 512-column halves of output tile t; one wide y DMA.
                # At the tail the exp stream is done: reuse the (now idle)
                # ps_s banks for double the psum ring, and issue the y DMA
                # from the idle ACT hwdge queue instead of SP's.
                yt = ypool.tile([128, 1024], F16, tag="yout", name=f"yt{t}_r{R[0]}")
                if tail:
                    pswide = ps_s.tile([128, 1024], FP, tag="s",
                                       name=f"psyt{t}_r{R[0]}")
                for nch in range(2):
                    if tail:
                        ps = pswide[:, nch * 512 : nch * 512 + 512]
                    else:
                        ps = ps_y.tile([128, 512], FP, tag="ps_proj",
                                       name=f"psy{t}_{nch}_r{R[0]}")[:]
                    for pair in range(2):
                        nc.tensor.matmul(
                            ps,
                            lhsT=otstc[pair][t // 4][
                                :, (t % 4) * 128 : (t % 4) * 128 + 128
                            ],
                            rhs=wo[pair][:, nch * 512 : nch * 512 + 512],
                            start=(pair == 0),
                            stop=(pair == 1),
                        )
                    # Pool can't read PSUM; at the tail ACT is idle so
                    # alternate DVE/ACT to pipeline back-to-back units
                    if tail and nch == 1:
                        nc.scalar.copy(yt[:, nch * 512 : nch * 512 + 512], ps)
                    else:
                        nc.vector.tensor_copy(
                            yt[:, nch * 512 : nch * 512 + 512], ps
                        )
                eng = nc.scalar if tail else nc.sync
                eng.dma_start(
                    y_d[t * 128 : (t + 1) * 128, :],
                    yt[:],
                )

            proj_units = []

            def maybe_fill():
                # keep the PE stream dense with next-chunk projection units;
                # yproj fillers wait for the normalize-boundary PE idle
                if proj_units:
                    proj_units.pop(0)()

            def boundary_fill(n=2):
                for _ in range(n):
                    if proj_units:
                        proj_units.pop(0)()
                    elif fillers:
                        fillers.pop(0)()

            def drain_proj_units():
                while proj_units:
                    proj_units.pop(0)()

            def queue_proj_chunk(c, x8t):
                # the xb DMA rides in the unit queue after the QK units so
                # the small S-gating repack DMAs aren't stuck behind the
                # bulk bf16 x transfer on the serial SP queue
                for wts, tpb, nm in ((wq_big, 0, "q"), (wk_big, 2, "k")):
                    for pair in range(2):
                        proj_units.append(
                            lambda w=wts, tb=tpb, n=nm, p=pair: emit_proj_qk1(
                                w, tb, n, c, p, x8t
                            )
                        )
                proj_units.append(lambda: emit_repack(c))
                xbt_box = []
                proj_units.append(
                    lambda: xbt_box.append(dma_xb(c, f"xbc{c}_r{R[0]}"))
                )
                for pair in range(2):
                    # the two halves must stay adjacent in the queue: they
                    # share one ps_proj ring slot across both units
                    proj_units.append(
                        lambda p=pair: emit_proj_v1(c, p, xbt_box[0], half=0)
                    )
                    proj_units.append(
                        lambda p=pair: emit_proj_v1(c, p, xbt_box[0], half=1)
                    )
                for pair in range(2):
                    for t in range(4 * c, 4 * c + 4):
                        proj_units.append(
                            lambda p=pair, t=t: emit_transpose1(p, t)
                        )

            def se_step(I, pair, h, jb, store, dr=None):
                # S matmuls + exp + mask for one (pair, h, jb); the p_sb
                # tile is parked in `store` for the lagging O stream
                hsl = slice(64 * h, 64 * h + 64)
                j0 = 2 * jb
                diag = j0 >= 4 * I
                zs = [max(0, (j0 + dj) * 128 - I * 512) for dj in range(2)]
                if dr is None:
                    dr = I >= 1
                s_ps = ps_s.tile([128, 1024], FP, tag="s",
                                 name=f"s{I}_{pair}_{h}_{jb}_r{R[0]}")
                for dj in range(2):
                    j = j0 + dj
                    z = zs[dj]
                    if dr:
                        # fp8 DoubleRow: K=64 packed as [32, 2] via repack
                        nc.tensor.matmul(
                            s_ps[:, dj * 512 + z : dj * 512 + 512],
                            lhsT=qkdr[j // 4][
                                32 * h : 32 * h + 32, :, 2 + pair,
                                (j % 4) * 128 : (j % 4) * 128 + 128,
                            ],
                            rhs=qkdr[I][32 * h : 32 * h + 32, :, pair, z:512],
                            start=True,
                            stop=True,
                            perf_mode=DR,
                        )
                    else:
                        nc.tensor.matmul(
                            s_ps[:, dj * 512 + z : dj * 512 + 512],
                            lhsT=qkTc[j // 4][
                                hsl, 2 + pair, (j % 4) * 128 : (j % 4) * 128 + 128
                            ],
                            rhs=qkTc[I][hsl, pair, z:512],
                            start=True,
                            stop=True,
                        )
                p_sb = ppool.tile([128, 1024], BF, tag="p",
                                  name=f"p{I}_{pair}_{h}_{jb}_r{R[0]}")
                if not diag:
                    nc.scalar.activation(p_sb[:], s_ps[:], EXP, scale=SCALE)
                else:
                    # trimmed: columns below the causal boundary were never
                    # computed
                    for dj in range(2):
                        lo = dj * 512 + zs[dj]
                        hi = dj * 512 + 512
                        nc.scalar.activation(
                            p_sb[:, lo:hi], s_ps[:, lo:hi], EXP, scale=SCALE
                        )
                for dj in range(2):
                    j = j0 + dj
                    z = zs[dj]
                    if j >= 4 * I:
                        # triangular strip at the causal boundary
                        ssl2 = slice(dj * 512 + z, dj * 512 + z + 128)
                        nc.gpsimd.tensor_mul(
                            p_sb[:, ssl2], p_sb[:, ssl2], mask[:, 384:512]
                        )
                store.append((zs, p_sb))

            def o_step(I, pair, h, jb, oT_h, store):
                jmax = 4 * I + 4
                zs, p_sb = store[jb]
                j0 = 2 * jb
                for dj in range(2):
                    j = j0 + dj
                    z = zs[dj]
                    nc.tensor.matmul(
                        oT_h[:, z:512],
                        lhsT=vaug[pair][j][:, 65 * h : 65 * h + 65],
                        rhs=p_sb[:, dj * 512 + z : dj * 512 + 512],
                        start=(j == 0),
                        stop=(j == jmax - 1),
                    )
                maybe_fill()

            def norm_step(I, pair, h, oT_h):
                # normalize O^T[0:64] by 1/rowsum into the stacked chunk
                recip = spool.tile([1, 512], FP, tag="recip",
                                   name=f"rc{I}_{pair}_{h}_r{R[0]}")
                nc.vector.reciprocal(recip[:], oT_h[64:65, :])
                bcast = spool.tile([64, 512], FP, tag="bcast",
                                   name=f"bc{I}_{pair}_{h}_r{R[0]}")
                nc.gpsimd.partition_broadcast(bcast[:], recip[:])
                if h == 0:
                    nc.vector.tensor_mul(
                        otstc[pair][I][0:64, :], oT_h[0:64, :], bcast[:]
                    )
                else:
                    onrm = spool.tile([64, 512], BF, tag="onrm",
                                      name=f"on{I}_{pair}_r{R[0]}")
                    nc.vector.tensor_mul(onrm[:], oT_h[0:64, :], bcast[:])
                    # partition shift 0->64 needs a DMA
                    nc.sync.dma_start(otstc[pair][I][64:128, :], onrm[:])
                boundary_fill()

            def att_seq(I):
                njb = (4 * I + 4) // 2
                return [
                    (pair, h, jb)
                    for pair in (0, 1)
                    for h in (1, 0)
                    for jb in range(njb)
                ]

            def emit_attention(I, lag=8, pre=0, stores=None, next_se=None):
                # lagged software pipeline: the O stream trails the S/exp
                # stream by `lag` jb-steps so O-side dependency stalls
                # (vaug availability, oT ring, normalize) never starve ACT.
                # Once this attention's SE stream is exhausted, the first
                # `next_se` steps of the NEXT attention's SE stream slot in,
                # bridging the exp stream across the phase boundary.
                njb = (4 * I + 4) // 2
                seq = att_seq(I)
                stores = stores if stores is not None else {}
                next_ptr = 0
                if next_se is not None:
                    nI, nstores, ncnt = next_se
                    nseq = att_seq(nI)
                oTs = {}
                s_ptr = pre
                for k in range(len(seq)):
                    while s_ptr < min(len(seq), k + lag + 1):
                        pair, h, jb = seq[s_ptr]
                        se_step(I, pair, h, jb,
                                stores.setdefault((pair, h), []))
                        s_ptr += 1
                    if (
                        s_ptr >= len(seq)
                        and next_se is not None
                        and next_ptr < ncnt
                    ):
                        pair, h, jb = nseq[next_ptr]
                        se_step(nI, pair, h, jb,
                                nstores.setdefault((pair, h), []),
                                dr=(nI >= 2))
                        next_ptr += 1
                    pair, h, jb = seq[k]
                    if jb == 0:
                        oTs[(pair, h)] = ps_o.tile(
                            [65, 512], FP, tag="oT",
                            name=f"o{I}_{pair}_{h}_r{R[0]}",
                        )
                    o_step(I, pair, h, jb, oTs[(pair, h)][:],
                           stores[(pair, h)])
                    if jb == njb - 1:
                        norm_step(I, pair, h, oTs[(pair, h)][:])
                while next_se is not None and next_ptr < ncnt:
                    pair, h, jb = nseq[next_ptr]
                    se_step(nI, pair, h, jb,
                            nstores.setdefault((pair, h), []))
                    next_ptr += 1
                for t in range(4 * I, 4 * I + 4):
                    fillers.append(
                        lambda t=t, **kw: emit_yproj_chunk(t, **kw)
                    )

            # ---- interleaved emission: QK projections for chunks 0-1 lead
            # so attention(1)'s exp stream starts as early as possible, then
            # proj chunk c + attention I=c; attention I=0 (shortest) last ----
            for rep in range(repeats):
                R[0] = rep
                if rep == 0:
                    x8t0, x8t1 = x8_by_chunk[0], x8_by_chunk[1]
                else:
                    x8t0 = dma_x8(0, f"x8c0_r{rep}")
                    x8t1 = dma_x8(1, f"x8c1_r{rep}")
                emit_proj_qk(0, x8t0)
                if rep == 0:
                    nc.sync.dma_start(mask[:], mask_d[:])
                    nc.sync.dma_start(ones_sb[:], ones_d[:])
                    nc.sync.dma_start(ident[:], ident_d[:])
                xbt0 = dma_xb(0, f"xbc0_r{rep}")
                xbt1 = dma_xb(1, f"xbc1_r{rep}")
                # chunk-1 QK + chunk-0 V/transposes ride as PE fillers in
                # attention(0)'s paced S/exp window
                for pair in range(2):
                    for wts, tpb, nm in ((wq_big, 0, "q"), (wk_big, 2, "k")):
                        proj_units.append(
                            lambda w=wts, tb=tpb, n=nm, p=pair: emit_proj_qk1(
                                w, tb, n, 1, p, x8t1
                            )
                        )
                proj_units.append(lambda: emit_repack(1))
                for pair in range(2):
                    proj_units.append(
                        lambda p=pair: emit_proj_v1(0, p, xbt0, half=0)
                    )
                    proj_units.append(
                        lambda p=pair: emit_proj_v1(0, p, xbt0, half=1)
                    )
                for pair in range(2):
                    for t in range(0, 4):
                        proj_units.append(
                            lambda p=pair, t=t: emit_transpose1(p, t)
                        )
                # attention(0) S+exp right away: no V dependency, so the
                # exp stream starts while the V projections wait on DMA
                se0 = {}
                for pair in range(2):
                    for h in (1, 0):
                        se0[(pair, h)] = st = []
                        for jb in range(2):
                            se_step(0, pair, h, jb, st)
                            maybe_fill()
                drain_proj_units()
                if rep == 0:
                    nc.sync.dma_start(
                        wo_big[:, :, :],
                        wob_d.rearrange("(n p) d -> p n d", p=128),
                    )
                # repacks feed S(2)+ only (attention(0/1) stay on the
                # plain-S path), so they can sit behind the bulk transfers
                emit_repack(0, k_only=True)
                emit_repack(1)
                # chunk-1 V/transposes pop during attention(0)'s O loop,
                # which also pre-issues attention(1)'s SE stream
                for pair in range(2):
                    proj_units.append(
                        lambda p=pair: emit_proj_v1(1, p, xbt1, half=0)
                    )
                    proj_units.append(
                        lambda p=pair: emit_proj_v1(1, p, xbt1, half=1)
                    )
                for pair in range(2):
                    for t in range(4, 8):
                        proj_units.append(
                            lambda p=pair, t=t: emit_transpose1(p, t)
                        )
                se1 = {}
                emit_attention(0, lag=0, pre=8, stores=se0,
                               next_se=(1, se1, 8))
                drain_proj_units()
                # chunk c+1 projections ride along as PE fillers inside
                # attention(c) so the exp stream never hits a proj wall;
                # each attention's O-tail pre-issues the next SE stream
                queue_proj_chunk(2, dma_x8(2, f"x8c2_r{R[0]}"))
                se2 = {}
                emit_attention(1, pre=8, stores=se1, next_se=(2, se2, 6))
                drain_proj_units()
                queue_proj_chunk(3, dma_x8(3, f"x8c3_r{R[0]}"))
                se3 = {}
                emit_attention(2, pre=6, stores=se2, next_se=(3, se3, 6))
                drain_proj_units()
                emit_attention(3, pre=6, stores=se3)
                while fillers:
                    fillers.pop(0)(tail=True)

    nc.compile()
    _nc_cache[key] = nc
    return nc


def make_in_maps(x, Wq, Wk, Wv, Wo):
    x = np.asarray(x, dtype=np.float32)
    Wq = np.asarray(Wq, dtype=np.float32)
    Wk = np.asarray(Wk, dtype=np.float32)
    Wv = np.asarray(Wv, dtype=np.float32)
    Wo = np.asarray(Wo, dtype=np.float32)
    in_maps = []
    for c in range(N_CORES):
        b, hg = c // 4, c % 4
        sl = slice(256 * hg, 256 * hg + 256)
        xT = np.ascontiguousarray(x[b].T)
        in_maps.append(
            {
                "xT8": xT.astype(NPF8),
                "xTb": xT.astype(NPBF),
                "wq8": np.ascontiguousarray((32.0 * Wq[sl, :]).T).astype(NPF8),
                "wk8": np.ascontiguousarray((32.0 * Wk[sl, :]).T).astype(NPF8),
                "wvb": np.ascontiguousarray(Wv[sl, :].T).astype(NPBF),
                "wob": np.ascontiguousarray(Wo[:, sl].T).astype(NPBF),
            }
        )
    return in_maps


def run_spmd(in_maps, trace=False, repeats=1, **kw):
    nc = build_kernel(repeats)
    return run_bass_kernel_spmd(nc, in_maps, list(range(N_CORES)), trace=trace, **kw)


def gather(results, bo):
    bo = np.asarray(bo, dtype=np.float32)
    y = np.empty((B, T, C), dtype=np.float32)
    for b in range(B):
        acc = results[4 * b]["y"].astype(np.float32).copy()
        for g in range(1, 4):
            acc += results[4 * b + g]["y"]
        y[b] = acc + bo[None, :]
    return y


def kernel(x, Wq, Wk, Wv, Wo, bo):
    res = run_spmd(make_in_maps(x, Wq, Wk, Wv, Wo))
    return gather(res.results, bo)
